# revision 11
# baseline (speedup 1.0000x reference)
"""Trainium2 Bass kernel for nn_KANSplineLayer (KAN spline layer, 8-core SPMD).

Math rewrite (same as v0, validated): the 9-triangle spline per (o,i) is a
continuous piecewise-linear function of t = 4*minmax(x) in [0,4], so it equals
a combination of {t, relu(t-1), relu(t-2), relu(t-3), 1}.  Spline contraction
shrinks from 256*9 to 256*4 (+rank-1 bias).

v1 restructure (everything in fp16 on device, PSUM accum fp32):
  - host casts x to fp16; x^T loaded with DMA XBAR transpose straight from
    DRAM (no PE transposes, no stage copies).
  - min/max partials per DMA segment; AllReduce(min) of [min|-max] fired
    ~60us earlier than v0.
  - output kept transposed ([outs, rows]) so BN/silu biases are per-partition
    (free via ACT bias); base path GEMM+silu runs during the collective.
  - weights stationary in PE, planes/x moving with N=512 => 1 cyc/row fp16.
  - fp16 output (host re-transposes + casts), halves output DMA bytes.
"""
import numpy as np

import concourse.bacc as bacc
import concourse.bass as bass
import concourse.tile as tile
from concourse import mybir
from concourse.bass_utils import run_bass_kernel_spmd

# ---- problem constants (hardcoded; kernel.py must be self-contained) ----
IN_F, OUT_F = 256, 256
K_KNOTS = 9
EPS_MINMAX = 1e-7
EPS_BN = 1e-3
B, H, W = 32, 32, 32
N_TOTAL = B * H * W            # 32768 rows
N_CORES = 8
N_SHARD = N_TOTAL // N_CORES   # 4096 rows per core
CH = 512                       # phase-2 row chunk (moving dim)
N_CHUNKS = N_SHARD // CH       # 8
N_SEG = 4                      # x^T DMA-transpose segments (per feature block: 2)
SEG = N_SHARD // 2             # 2048 rows per (block, half) segment

F32 = mybir.dt.float32
F16 = mybir.dt.float16
USE_RDMA = True


def _host_prep(base_weight, spline_weight, spline_scaler,
               bn_base_gamma, bn_base_beta, bn_base_mean, bn_base_var,
               bn_spline_gamma, bn_spline_beta, bn_spline_mean, bn_spline_var):
    """Fold BN + rewrite spline into relu-plane weights. All in float64.

    Returns SBUF-layout arrays:
      w_t  [128, 2, 2, 128]   (i', b, bo, o')  t-plane weights
      w_r  [128, 3, 2, 2, 128] (i', m, b, bo, o') relu-plane weights
      w_b  [128, 2, 2, 128]   base weights
      cs   [1, 256]           spline constant (bo-major)
      bb   [128, 2]           base bias per (o', bo), f32
    """
    f64 = np.float64
    w = np.asarray(spline_weight, f64) * np.asarray(spline_scaler, f64)[:, :, None]
    knots = np.linspace(-1.0, 1.0, K_KNOTS).astype(f64)
    jg = np.arange(5, dtype=f64) / 4.0
    tri = np.maximum(0.0, 1.0 - np.abs(jg[None, :] - knots[:, None]))   # [k, j]
    G = np.einsum('oik,kj->oij', w, tri)                                # [o,i,5]
    a_s = np.asarray(bn_spline_gamma, f64) / np.sqrt(np.asarray(bn_spline_var, f64) + EPS_BN)
    b_s = np.asarray(bn_spline_beta, f64) - a_s * np.asarray(bn_spline_mean, f64)
    G = G * a_s[:, None, None]
    W_t = (G[:, :, 1] - G[:, :, 0]).T                                   # [i,o]
    Hs = [(G[:, :, 2] - 2 * G[:, :, 1] + G[:, :, 0]).T,
          (G[:, :, 3] - 2 * G[:, :, 2] + G[:, :, 1]).T,
          (G[:, :, 4] - 2 * G[:, :, 3] + G[:, :, 2]).T]                 # [i,o]
    C_s = G[:, :, 0].sum(axis=1) + b_s                                  # [o]
    a_b = np.asarray(bn_base_gamma, f64) / np.sqrt(np.asarray(bn_base_var, f64) + EPS_BN)
    b_b = np.asarray(bn_base_beta, f64) - a_b * np.asarray(bn_base_mean, f64)
    Wb = np.asarray(base_weight, f64) * a_b[None, :]                    # [i,o]

    def blk(M):  # [in, out] f64 -> [128, 2(b), 2(bo), 128] (i', b, bo, o')
        return M.reshape(2, 128, 2, 128).transpose(1, 0, 2, 3)

    w_t = blk(W_t).astype(np.float16)
    w_b = blk(Wb).astype(np.float16)
    w_r = np.stack([blk(Hm) for Hm in Hs], axis=1).astype(np.float16)  # [128,3,2,2,128]
    cs = C_s.astype(np.float16)[None, :]                               # [1, 256]
    bb = b_b.reshape(2, 128).T.astype(np.float32)                      # [128, 2]
    return w_t, w_r, w_b, cs, bb


def _build_bass(use_pool_epilogue=False):
    nc = bacc.Bacc(num_devices=N_CORES)
    x_sh = nc.declare_dram_parameter("x_sh", [N_SHARD, IN_F], F16, isOutput=False)
    w_t_d = nc.declare_dram_parameter("w_t", [128, 2, 2, 128], F16, isOutput=False)
    w_r_d = nc.declare_dram_parameter("w_r", [128, 3, 2, 2, 128], F16, isOutput=False)
    w_b_d = nc.declare_dram_parameter("w_b", [128, 2, 2, 128], F16, isOutput=False)
    cs_d = nc.declare_dram_parameter("cs", [1, 256], F16, isOutput=False)
    bb_d = nc.declare_dram_parameter("bb", [128, 2], F32, isOutput=False)
    out_t = nc.declare_dram_parameter("out_t", [2, 128, N_SHARD], F16, isOutput=True)

    from contextlib import ExitStack
    with tile.TileContext(nc) as tc, ExitStack() as es:
        cons = es.enter_context(tc.tile_pool(name="cons", bufs=1))
        silu_p = es.enter_context(tc.tile_pool(name="silu", bufs=1))
        planes_p = es.enter_context(tc.tile_pool(name="planes", bufs=2))
        psS = es.enter_context(tc.tile_pool(name="psS", bufs=4, space="PSUM"))
        psB = es.enter_context(tc.tile_pool(name="psB", bufs=2, space="PSUM"))
        outp = es.enter_context(tc.tile_pool(name="outp", bufs=3))
        dram = es.enter_context(tc.tile_pool(name="dram", bufs=2, space="DRAM"))

        # ---- weight loads (scalar/ACT queue; x transposes go on sync) ----
        wb_sb = cons.tile([128, 2, 2, 128], F16, name="wb_sb")
        nc.scalar.dma_start(out=wb_sb[:], in_=w_b_d[:])
        wt_sb = cons.tile([128, 2, 2, 128], F16, name="wt_sb")
        nc.scalar.dma_start(out=wt_sb[:], in_=w_t_d[:])
        wr_sb = cons.tile([128, 3, 2, 2, 128], F16, name="wr_sb")
        nc.scalar.dma_start(out=wr_sb[:], in_=w_r_d[:])
        cs_sb = cons.tile([1, 256], F16, name="cs_sb")
        nc.scalar.dma_start(out=cs_sb[:], in_=cs_d[:])
        bb_sb = cons.tile([128, 2], F32, name="bb_sb")
        nc.scalar.dma_start(out=bb_sb[:], in_=bb_d[:])

        ones_f32 = cons.tile([1, CH], F32, name="ones_f32")
        nc.vector.memset(ones_f32[:], 1.0)
        ones = cons.tile([1, CH], F16, name="ones")
        nc.vector.tensor_copy(out=ones[:], in_=ones_f32[:])
        rb = cons.tile([128, 2], F32, name="rb")     # ACT Relu biases -1, -2
        nc.vector.memset(rb[:, 0:1], -1.0)
        nc.vector.memset(rb[:, 1:2], -2.0)

        # ---- phase 1: x^T via DMA XBAR transpose, segmented minmax ----
        xt = cons.tile([128, 2, N_SHARD], F16, name="xt")
        for b in range(2):
            for h in range(2):
                nc.sync.dma_start(
                    out=xt[:, b, h * SEG:(h + 1) * SEG],
                    in_=x_sh[h * SEG:(h + 1) * SEG, b * 128:(b + 1) * 128],
                    transpose=True)

        # local min/max: block 0 reduces on Pool, block 1 on DVE (parallel)
        pmin = cons.tile([128, 2, 2], F32, name="pmin")   # (b, h)
        pmax = cons.tile([128, 2, 2], F32, name="pmax")
        for b in range(2):
            eng = nc.vector
            for h in range(2):
                seg = xt[:, b, h * SEG:(h + 1) * SEG]
                eng.tensor_reduce(
                    out=pmin[:, b, h:h + 1], in_=seg,
                    op=mybir.AluOpType.min, axis=mybir.AxisListType.X)
                eng.tensor_reduce(
                    out=pmax[:, b, h:h + 1], in_=seg,
                    op=mybir.AluOpType.max, axis=mybir.AxisListType.X)
        mm_loc = cons.tile([128, 4], F32, name="mm_loc")  # [min0,min1,-max0,-max1]
        tmax = cons.tile([128, 2], F32, name="tmax")
        nc.vector.tensor_tensor(
            out=mm_loc[:, 0:2], in0=pmin[:, :, 0], in1=pmin[:, :, 1],
            op=mybir.AluOpType.min)
        nc.vector.tensor_tensor(
            out=tmax[:], in0=pmax[:, :, 0], in1=pmax[:, :, 1],
            op=mybir.AluOpType.max)
        nc.vector.tensor_scalar(
            out=mm_loc[:, 2:4], in0=tmax[:], scalar1=-1.0, scalar2=None,
            op0=mybir.AluOpType.mult)

        # ---- global min/max across the 8 cores ----
        if USE_RDMA:
            # XOR recursive doubling over the 8 same-device cores with raw
            # p2p remote DMA: no CC cores, no runtime collective barrier.
            # Round k: send acc to peer (tpb ^= 1<<k); wait for the peer's
            # symmetric write (+2 on rsem[k], one sem per round so a fast
            # core's later round can't satisfy an earlier wait); combine
            # with elementwise min.  [min|-max] carries max via min.
            # Hand-synced inside tile_critical (scheduler can't model the
            # cross-core sem arrivals).
            rsem = [nc.alloc_semaphore(f"xmm{k}") for k in range(3)]
            psem = nc.alloc_semaphore("xmm_prep")
            xch = nc.alloc_semaphore("xmm_comb")
            loc_sem = nc.alloc_semaphore("rdma_local")
            XW = 128   # pad payload to 512B/partition for the DMA ucode
            rx = [cons.tile([128, XW], F32, name=f"rx{k}") for k in range(3)]
            accs = [cons.tile([128, XW], F32, name=f"acc{k}") for k in range(4)]
            nc.vector.memset(accs[0][:, 4:XW], 0.0)
            nc.vector.tensor_copy(out=accs[0][:, 0:4], in_=mm_loc[:])
            with tc.tile_critical():
                for k in range(3):
                    rdests = [None] * 8
                    rdests[4 if (1 << k) & 4 else 0] = (0, 1 << k)
                    prep = nc.gpsimd.remote_dma_broadcast(
                        out_ap=rx[k][:], in_ap=accs[k][:],
                        remote_sem=rsem[k], local_sem=loc_sem,
                        rdests=rdests)
                    if k > 0:
                        prep._wait_ge(xch, k)  # Pool in-order: gates trigger too
                    prep.then_inc(psem, 1)
                    nc.gpsimd.trigger_dma(count=1)._wait_ge(psem, k + 1)
                    comb = nc.vector.tensor_tensor(
                        out=accs[k + 1][:], in0=accs[k][:], in1=rx[k][:],
                        op=mybir.AluOpType.min)
                    comb._wait_ge(rsem[k], 2)
                    comb.then_inc(xch, 1)
            gmm = accs[3]   # [gmin0,gmin1,-gmax0,-gmax1] in cols 0:4
        else:
            cc_in = dram.tile([128, 4], F32)
            cc_out = dram.tile([128, 4], F32)
            nc.sync.dma_start(out=cc_in[:], in_=mm_loc[:])
            nc.gpsimd.collective_compute(
                "AllReduce", mybir.AluOpType.min,
                replica_groups=[list(range(N_CORES))],
                ins=[cc_in.opt()], outs=[cc_out.opt()])
            gmm = cons.tile([128, 4], F32, name="gmm")
            nc.sync.dma_start(out=gmm[:], in_=cc_out[:])

        # s4 = 4/(gmax-gmin+eps)
        nrng = cons.tile([128, 2], F32, name="nrng")
        qt = cons.tile([128, 2], F32, name="qt")
        s4 = cons.tile([128, 2], F32, name="s4")
        nc.vector.tensor_tensor(
            out=nrng[:], in0=gmm[:, 0:2], in1=gmm[:, 2:4],
            op=mybir.AluOpType.add)                       # gmin - gmax
        nc.vector.tensor_scalar(
            out=qt[:], in0=nrng[:], scalar1=-0.25, scalar2=EPS_MINMAX * 0.25,
            op0=mybir.AluOpType.mult, op1=mybir.AluOpType.add)
        nc.vector.reciprocal(out=s4[:], in_=qt[:])

        # ---- phase 1.5: base path (x @ Wb, silu w/ per-partition bias) ----
        # runs on PE/ACT while the collective is in flight.
        silu_sb = [[silu_p.tile([128, CH], F16, name=f"silu_{c}_{bo}")
                    for bo in range(2)] for c in range(N_CHUNKS)]
        for c in range(N_CHUNKS):
            cs_sl = slice(c * CH, (c + 1) * CH)
            for bo in range(2):
                pb = psB.tile([128, CH], F32)
                for b in range(2):
                    nc.tensor.matmul(
                        pb[:], wb_sb[:, b, bo, :], xt[:, b, cs_sl],
                        start=(b == 0), stop=(b == 1), skip_group_check=True)
                nc.scalar.activation(
                    out=silu_sb[c][bo][:], in_=pb[:],
                    func=mybir.ActivationFunctionType.Silu,
                    bias=bb_sb[:, bo:bo + 1], scale=1.0)

        # ---- phase 2: planes + spline GEMMs + epilogue ----
        for c in range(N_CHUNKS):
            cs_sl = slice(c * CH, (c + 1) * CH)
            tpl = [planes_p.tile([128, CH], F16, tag=f"t{b}", name=f"t{b}_{c}")
                   for b in range(2)]
            rpl = [[planes_p.tile([128, CH], F16, tag=f"r{m}{b}", name=f"r{m}{b}_{c}")
                    for b in range(2)] for m in range(3)]
            for b in range(2):
                nc.vector.tensor_scalar(
                    out=tpl[b][:], in0=xt[:, b, cs_sl],
                    scalar1=gmm[:, b:b + 1], scalar2=s4[:, b:b + 1],
                    op0=mybir.AluOpType.subtract, op1=mybir.AluOpType.mult)
                for m in (1, 2):
                    nc.scalar.activation(
                        out=rpl[m - 1][b][:], in_=tpl[b][:],
                        func=mybir.ActivationFunctionType.Relu,
                        bias=rb[:, m - 1:m], scale=1.0)
                nc.vector.tensor_scalar(
                    out=rpl[2][b][:], in0=tpl[b][:], scalar1=3.0, scalar2=0.0,
                    op0=mybir.AluOpType.subtract, op1=mybir.AluOpType.max)
            for bo in range(2):
                ps = psS.tile([128, CH], F32)
                nc.tensor.matmul(
                    ps[:], cs_sb[:, bo * 128:(bo + 1) * 128], ones[:],
                    start=True, stop=False, skip_group_check=True)
                for b in range(2):
                    nc.tensor.matmul(
                        ps[:], wt_sb[:, b, bo, :], tpl[b][:],
                        start=False, stop=False, skip_group_check=True)
                for m in range(3):
                    for b in range(2):
                        nc.tensor.matmul(
                            ps[:], wr_sb[:, m, b, bo, :], rpl[m][b][:],
                            start=False, stop=(m == 2 and b == 1),
                            skip_group_check=True)
                o = outp.tile([128, CH], F16)
                eng = nc.gpsimd if use_pool_epilogue else nc.vector
                eng.tensor_tensor(
                    out=o[:], in0=ps[:], in1=silu_sb[c][bo][:],
                    op=mybir.AluOpType.add)
                nc.sync.dma_start(out=out_t[bo, :, cs_sl], in_=o[:])
    nc.compile()
    return nc


_CACHE = {}


def make_in_maps(inputs):
    x = np.asarray(inputs["x"], np.float32).reshape(N_TOTAL, IN_F)
    x16 = x.astype(np.float16)
    w_t, w_r, w_b, cs, bb = _host_prep(
        **{k: v for k, v in inputs.items() if k != "x"})
    return [{
        "x_sh": np.ascontiguousarray(x16[c * N_SHARD:(c + 1) * N_SHARD]),
        "w_t": w_t, "w_r": w_r, "w_b": w_b, "cs": cs, "bb": bb,
    } for c in range(N_CORES)]


def kernel(**inputs):
    if "nc" not in _CACHE:
        _CACHE["nc"] = _build_bass()
    nc = _CACHE["nc"]
    in_maps = make_in_maps(inputs)
    res = run_bass_kernel_spmd(nc, in_maps, list(range(N_CORES)))
    out = np.empty((N_TOTAL, OUT_F), np.float32)
    for c in range(N_CORES):
        ot = np.asarray(res.results[c]["out_t"], np.float32)  # [2,128,4096]
        out[c * N_SHARD:(c + 1) * N_SHARD] = (
            ot.transpose(2, 0, 1).reshape(N_SHARD, OUT_F))
    return out.reshape(B, H, W, OUT_F)


# revision 13
# speedup vs baseline: 81.1087x; 81.1087x over previous
"""Trainium2 Bass kernel for nn_KANSplineLayer (KAN spline layer, 8-core SPMD).

Math rewrite (same as v0, validated): the 9-triangle spline per (o,i) is a
continuous piecewise-linear function of t = 4*minmax(x) in [0,4], so it equals
a combination of {t, relu(t-1), relu(t-2), relu(t-3), 1}.  Spline contraction
shrinks from 256*9 to 256*4 (+rank-1 bias).

v1 restructure (everything in fp16 on device, PSUM accum fp32):
  - host casts x to fp16; x^T loaded with DMA XBAR transpose straight from
    DRAM (no PE transposes, no stage copies).
  - min/max partials per DMA segment; AllReduce(min) of [min|-max] fired
    ~60us earlier than v0.
  - output kept transposed ([outs, rows]) so BN/silu biases are per-partition
    (free via ACT bias); base path GEMM+silu runs during the collective.
  - weights stationary in PE, planes/x moving with N=512 => 1 cyc/row fp16.
  - fp16 output (host re-transposes + casts), halves output DMA bytes.
"""
import numpy as np

import concourse.bacc as bacc
import concourse.bass as bass
import concourse.tile as tile
from concourse import mybir
from concourse.bass_utils import run_bass_kernel_spmd

# ---- problem constants (hardcoded; kernel.py must be self-contained) ----
IN_F, OUT_F = 256, 256
K_KNOTS = 9
EPS_MINMAX = 1e-7
EPS_BN = 1e-3
B, H, W = 32, 32, 32
N_TOTAL = B * H * W            # 32768 rows
N_CORES = 8
N_SHARD = N_TOTAL // N_CORES   # 4096 rows per core
CH = 512                       # phase-2 row chunk (moving dim)
N_CHUNKS = N_SHARD // CH       # 8
N_SEG = 4                      # x^T DMA-transpose segments (per feature block: 2)
SEG = N_SHARD // 2             # 2048 rows per (block, half) segment

F32 = mybir.dt.float32
F16 = mybir.dt.float16
USE_RDMA = False


def _host_prep(base_weight, spline_weight, spline_scaler,
               bn_base_gamma, bn_base_beta, bn_base_mean, bn_base_var,
               bn_spline_gamma, bn_spline_beta, bn_spline_mean, bn_spline_var):
    """Fold BN + rewrite spline into relu-plane weights. All in float64.

    Returns SBUF-layout arrays:
      w_t  [128, 2, 2, 128]   (i', b, bo, o')  t-plane weights
      w_r  [128, 3, 2, 2, 128] (i', m, b, bo, o') relu-plane weights
      w_b  [128, 2, 2, 128]   base weights
      cs   [1, 256]           spline constant (bo-major)
      bb   [128, 2]           base bias per (o', bo), f32
    """
    f64 = np.float64
    w = np.asarray(spline_weight, f64) * np.asarray(spline_scaler, f64)[:, :, None]
    knots = np.linspace(-1.0, 1.0, K_KNOTS).astype(f64)
    jg = np.arange(5, dtype=f64) / 4.0
    tri = np.maximum(0.0, 1.0 - np.abs(jg[None, :] - knots[:, None]))   # [k, j]
    G = np.einsum('oik,kj->oij', w, tri)                                # [o,i,5]
    a_s = np.asarray(bn_spline_gamma, f64) / np.sqrt(np.asarray(bn_spline_var, f64) + EPS_BN)
    b_s = np.asarray(bn_spline_beta, f64) - a_s * np.asarray(bn_spline_mean, f64)
    G = G * a_s[:, None, None]
    W_t = (G[:, :, 1] - G[:, :, 0]).T                                   # [i,o]
    Hs = [(G[:, :, 2] - 2 * G[:, :, 1] + G[:, :, 0]).T,
          (G[:, :, 3] - 2 * G[:, :, 2] + G[:, :, 1]).T,
          (G[:, :, 4] - 2 * G[:, :, 3] + G[:, :, 2]).T]                 # [i,o]
    C_s = G[:, :, 0].sum(axis=1) + b_s                                  # [o]
    a_b = np.asarray(bn_base_gamma, f64) / np.sqrt(np.asarray(bn_base_var, f64) + EPS_BN)
    b_b = np.asarray(bn_base_beta, f64) - a_b * np.asarray(bn_base_mean, f64)
    Wb = np.asarray(base_weight, f64) * a_b[None, :]                    # [i,o]

    def blk(M):  # [in, out] f64 -> [128, 2(b), 2(bo), 128] (i', b, bo, o')
        return M.reshape(2, 128, 2, 128).transpose(1, 0, 2, 3)

    w_t = blk(W_t).astype(np.float16)
    w_b = blk(Wb).astype(np.float16)
    w_r = np.stack([blk(Hm) for Hm in Hs], axis=1).astype(np.float16)  # [128,3,2,2,128]
    cs = C_s.astype(np.float16)[None, :]                               # [1, 256]
    bb = b_b.reshape(2, 128).T.astype(np.float32)                      # [128, 2]
    return w_t, w_r, w_b, cs, bb


def _build_bass(use_pool_epilogue=False):
    nc = bacc.Bacc(num_devices=N_CORES)
    x_sh = nc.declare_dram_parameter("x_sh", [N_SHARD, IN_F], F16, isOutput=False)
    w_t_d = nc.declare_dram_parameter("w_t", [128, 2, 2, 128], F16, isOutput=False)
    w_r_d = nc.declare_dram_parameter("w_r", [128, 3, 2, 2, 128], F16, isOutput=False)
    w_b_d = nc.declare_dram_parameter("w_b", [128, 2, 2, 128], F16, isOutput=False)
    cs_d = nc.declare_dram_parameter("cs", [1, 256], F16, isOutput=False)
    bb_d = nc.declare_dram_parameter("bb", [128, 2], F32, isOutput=False)
    out_t = nc.declare_dram_parameter("out_t", [2, 128, N_SHARD], F16, isOutput=True)

    from contextlib import ExitStack
    with tile.TileContext(nc) as tc, ExitStack() as es:
        cons = es.enter_context(tc.tile_pool(name="cons", bufs=1))
        silu_p = es.enter_context(tc.tile_pool(name="silu", bufs=1))
        planes_p = es.enter_context(tc.tile_pool(name="planes", bufs=2))
        psS = es.enter_context(tc.tile_pool(name="psS", bufs=6, space="PSUM"))
        psB = es.enter_context(tc.tile_pool(name="psB", bufs=2, space="PSUM"))
        outp = es.enter_context(tc.tile_pool(name="outp", bufs=3))
        dram = es.enter_context(tc.tile_pool(name="dram", bufs=2, space="DRAM"))

        # ---- weight loads (scalar/ACT queue; x transposes go on sync) ----
        wb_sb = cons.tile([128, 2, 2, 128], F16, name="wb_sb")
        nc.scalar.dma_start(out=wb_sb[:], in_=w_b_d[:])
        wt_sb = cons.tile([128, 2, 2, 128], F16, name="wt_sb")
        nc.scalar.dma_start(out=wt_sb[:], in_=w_t_d[:])
        wr_sb = cons.tile([128, 3, 2, 2, 128], F16, name="wr_sb")
        nc.scalar.dma_start(out=wr_sb[:], in_=w_r_d[:])
        cs_sb = cons.tile([1, 256], F16, name="cs_sb")
        nc.scalar.dma_start(out=cs_sb[:], in_=cs_d[:])
        bb_sb = cons.tile([128, 2], F32, name="bb_sb")
        nc.scalar.dma_start(out=bb_sb[:], in_=bb_d[:])

        ones_f32 = cons.tile([1, CH], F32, name="ones_f32")
        nc.vector.memset(ones_f32[:], 1.0)
        ones = cons.tile([1, CH], F16, name="ones")
        nc.vector.tensor_copy(out=ones[:], in_=ones_f32[:])
        rb = cons.tile([128, 2], F32, name="rb")     # ACT Relu biases -1, -2
        nc.vector.memset(rb[:, 0:1], -1.0)
        nc.vector.memset(rb[:, 1:2], -2.0)

        # ---- phase 1: x^T via DMA XBAR transpose, segmented minmax ----
        xt = cons.tile([128, 2, N_SHARD], F16, name="xt")
        for b in range(2):
            for h in range(2):
                nc.sync.dma_start(
                    out=xt[:, b, h * SEG:(h + 1) * SEG],
                    in_=x_sh[h * SEG:(h + 1) * SEG, b * 128:(b + 1) * 128],
                    transpose=True)

        # local min/max: block 0 reduces on Pool, block 1 on DVE (parallel)
        pmin = cons.tile([128, 2, 2], F32, name="pmin")   # (b, h)
        pmax = cons.tile([128, 2, 2], F32, name="pmax")
        for b in range(2):
            eng = nc.vector
            for h in range(2):
                seg = xt[:, b, h * SEG:(h + 1) * SEG]
                eng.tensor_reduce(
                    out=pmin[:, b, h:h + 1], in_=seg,
                    op=mybir.AluOpType.min, axis=mybir.AxisListType.X)
                eng.tensor_reduce(
                    out=pmax[:, b, h:h + 1], in_=seg,
                    op=mybir.AluOpType.max, axis=mybir.AxisListType.X)
        mm_loc = cons.tile([128, 4], F32, name="mm_loc")  # [min0,min1,-max0,-max1]
        tmax = cons.tile([128, 2], F32, name="tmax")
        nc.vector.tensor_tensor(
            out=mm_loc[:, 0:2], in0=pmin[:, :, 0], in1=pmin[:, :, 1],
            op=mybir.AluOpType.min)
        nc.vector.tensor_tensor(
            out=tmax[:], in0=pmax[:, :, 0], in1=pmax[:, :, 1],
            op=mybir.AluOpType.max)
        nc.vector.tensor_scalar(
            out=mm_loc[:, 2:4], in0=tmax[:], scalar1=-1.0, scalar2=None,
            op0=mybir.AluOpType.mult)

        # ---- global min/max across the 8 cores ----
        if USE_RDMA:
            # XOR recursive doubling over the 8 same-device cores with raw
            # p2p remote DMA: no CC cores, no runtime collective barrier.
            # Round k: send acc to peer (tpb ^= 1<<k); wait for the peer's
            # symmetric write (+2 on rsem[k], one sem per round so a fast
            # core's later round can't satisfy an earlier wait); combine
            # with elementwise min.  [min|-max] carries max via min.
            # Hand-synced inside tile_critical (scheduler can't model the
            # cross-core sem arrivals).
            rsem = [nc.alloc_semaphore(f"xmm{k}") for k in range(3)]
            psem = nc.alloc_semaphore("xmm_prep")
            xch = nc.alloc_semaphore("xmm_comb")
            loc_sem = nc.alloc_semaphore("rdma_local")
            XW = 128   # pad payload to 512B/partition for the DMA ucode
            rx = [cons.tile([128, XW], F32, name=f"rx{k}") for k in range(3)]
            accs = [cons.tile([128, XW], F32, name=f"acc{k}") for k in range(4)]
            nc.vector.memset(accs[0][:, 4:XW], 0.0)
            nc.vector.tensor_copy(out=accs[0][:, 0:4], in_=mm_loc[:])
            with tc.tile_critical():
                for k in range(3):
                    rdests = [None] * 8
                    rdests[4 if (1 << k) & 4 else 0] = (0, 1 << k)
                    prep = nc.gpsimd.remote_dma_broadcast(
                        out_ap=rx[k][:], in_ap=accs[k][:],
                        remote_sem=rsem[k], local_sem=loc_sem,
                        rdests=rdests)
                    if k > 0:
                        prep._wait_ge(xch, k)  # Pool in-order: gates trigger too
                    prep.then_inc(psem, 1)
                    nc.gpsimd.trigger_dma(count=1)._wait_ge(psem, k + 1)
                    comb = nc.vector.tensor_tensor(
                        out=accs[k + 1][:], in0=accs[k][:], in1=rx[k][:],
                        op=mybir.AluOpType.min)
                    comb._wait_ge(rsem[k], 2)
                    comb.then_inc(xch, 1)
            gmm = accs[3]   # [gmin0,gmin1,-gmax0,-gmax1] in cols 0:4
        else:
            cc_in = dram.tile([128, 4], F32)
            cc_out = dram.tile([128, 4], F32)
            nc.sync.dma_start(out=cc_in[:], in_=mm_loc[:])
            nc.gpsimd.collective_compute(
                "AllReduce", mybir.AluOpType.min,
                replica_groups=[list(range(N_CORES))],
                ins=[cc_in.opt()], outs=[cc_out.opt()])
            gmm = cons.tile([128, 4], F32, name="gmm")
            nc.sync.dma_start(out=gmm[:], in_=cc_out[:])

        # s4 = 4/(gmax-gmin+eps)
        nrng = cons.tile([128, 2], F32, name="nrng")
        qt = cons.tile([128, 2], F32, name="qt")
        s4 = cons.tile([128, 2], F32, name="s4")
        nc.vector.tensor_tensor(
            out=nrng[:], in0=gmm[:, 0:2], in1=gmm[:, 2:4],
            op=mybir.AluOpType.add)                       # gmin - gmax
        nc.vector.tensor_scalar(
            out=qt[:], in0=nrng[:], scalar1=-0.25, scalar2=EPS_MINMAX * 0.25,
            op0=mybir.AluOpType.mult, op1=mybir.AluOpType.add)
        nc.vector.reciprocal(out=s4[:], in_=qt[:])

        # ---- phase 1.5: base path (x @ Wb, silu w/ per-partition bias) ----
        # runs on PE/ACT while the collective is in flight.
        silu_sb = [[silu_p.tile([128, CH], F16, name=f"silu_{c}_{bo}")
                    for bo in range(2)] for c in range(N_CHUNKS)]
        for c in range(N_CHUNKS):
            cs_sl = slice(c * CH, (c + 1) * CH)
            for bo in range(2):
                pb = psB.tile([128, CH], F32)
                for b in range(2):
                    nc.tensor.matmul(
                        pb[:], wb_sb[:, b, bo, :], xt[:, b, cs_sl],
                        start=(b == 0), stop=(b == 1), skip_group_check=True)
                nc.scalar.activation(
                    out=silu_sb[c][bo][:], in_=pb[:],
                    func=mybir.ActivationFunctionType.Silu,
                    bias=bb_sb[:, bo:bo + 1], scale=1.0)

        # ---- phase 2: planes + spline GEMMs + epilogue ----
        for c in range(N_CHUNKS):
            cs_sl = slice(c * CH, (c + 1) * CH)
            tpl = [planes_p.tile([128, CH], F16, tag=f"t{b}", name=f"t{b}_{c}")
                   for b in range(2)]
            rpl = [[planes_p.tile([128, CH], F16, tag=f"r{m}{b}", name=f"r{m}{b}_{c}")
                    for b in range(2)] for m in range(3)]
            for b in range(2):
                nc.vector.tensor_scalar(
                    out=tpl[b][:], in0=xt[:, b, cs_sl],
                    scalar1=gmm[:, b:b + 1], scalar2=s4[:, b:b + 1],
                    op0=mybir.AluOpType.subtract, op1=mybir.AluOpType.mult)
                for m in (1, 2):
                    nc.scalar.activation(
                        out=rpl[m - 1][b][:], in_=tpl[b][:],
                        func=mybir.ActivationFunctionType.Relu,
                        bias=rb[:, m - 1:m], scale=1.0)
                nc.vector.tensor_scalar(
                    out=rpl[2][b][:], in0=tpl[b][:], scalar1=3.0, scalar2=0.0,
                    op0=mybir.AluOpType.subtract, op1=mybir.AluOpType.max)
            for bo in range(2):
                ps = psS.tile([128, CH], F32)
                nc.tensor.matmul(
                    ps[:], cs_sb[:, bo * 128:(bo + 1) * 128], ones[:],
                    start=True, stop=False, skip_group_check=True)
                for b in range(2):
                    nc.tensor.matmul(
                        ps[:], wt_sb[:, b, bo, :], tpl[b][:],
                        start=False, stop=False, skip_group_check=True)
                for m in range(3):
                    for b in range(2):
                        nc.tensor.matmul(
                            ps[:], wr_sb[:, m, b, bo, :], rpl[m][b][:],
                            start=False, stop=(m == 2 and b == 1),
                            skip_group_check=True)
                o = outp.tile([128, CH], F16)
                eng = nc.gpsimd if use_pool_epilogue else nc.vector
                eng.tensor_tensor(
                    out=o[:], in0=ps[:], in1=silu_sb[c][bo][:],
                    op=mybir.AluOpType.add)
                nc.sync.dma_start(out=out_t[bo, :, cs_sl], in_=o[:])
    nc.compile()
    return nc


_CACHE = {}


def make_in_maps(inputs):
    x = np.asarray(inputs["x"], np.float32).reshape(N_TOTAL, IN_F)
    x16 = x.astype(np.float16)
    w_t, w_r, w_b, cs, bb = _host_prep(
        **{k: v for k, v in inputs.items() if k != "x"})
    return [{
        "x_sh": np.ascontiguousarray(x16[c * N_SHARD:(c + 1) * N_SHARD]),
        "w_t": w_t, "w_r": w_r, "w_b": w_b, "cs": cs, "bb": bb,
    } for c in range(N_CORES)]


def kernel(**inputs):
    if "nc" not in _CACHE:
        _CACHE["nc"] = _build_bass()
    nc = _CACHE["nc"]
    in_maps = make_in_maps(inputs)
    res = run_bass_kernel_spmd(nc, in_maps, list(range(N_CORES)))
    out = np.empty((N_TOTAL, OUT_F), np.float32)
    for c in range(N_CORES):
        ot = np.asarray(res.results[c]["out_t"], np.float32)  # [2,128,4096]
        out[c * N_SHARD:(c + 1) * N_SHARD] = (
            ot.transpose(2, 0, 1).reshape(N_SHARD, OUT_F))
    return out.reshape(B, H, W, OUT_F)


# revision 19
# speedup vs baseline: 82.3722x; 1.0156x over previous
"""Trainium2 Bass kernel for nn_KANSplineLayer (KAN spline layer, 8-core SPMD).

Math rewrite (same as v0, validated): the 9-triangle spline per (o,i) is a
continuous piecewise-linear function of t = 4*minmax(x) in [0,4], so it equals
a combination of {t, relu(t-1), relu(t-2), relu(t-3), 1}.  Spline contraction
shrinks from 256*9 to 256*4 (+rank-1 bias).

v1 restructure (everything in fp16 on device, PSUM accum fp32):
  - host casts x to fp16; x^T loaded with DMA XBAR transpose straight from
    DRAM (no PE transposes, no stage copies).
  - min/max partials per DMA segment; AllReduce(min) of [min|-max] fired
    ~60us earlier than v0.
  - output kept transposed ([outs, rows]) so BN/silu biases are per-partition
    (free via ACT bias); base path GEMM+silu runs during the collective.
  - weights stationary in PE, planes/x moving with N=512 => 1 cyc/row fp16.
  - fp16 output (host re-transposes + casts), halves output DMA bytes.
"""
import numpy as np

import concourse.bacc as bacc
import concourse.bass as bass
import concourse.tile as tile
from concourse import mybir
from concourse.bass_utils import run_bass_kernel_spmd

# ---- problem constants (hardcoded; kernel.py must be self-contained) ----
IN_F, OUT_F = 256, 256
K_KNOTS = 9
EPS_MINMAX = 1e-7
EPS_BN = 1e-3
B, H, W = 32, 32, 32
N_TOTAL = B * H * W            # 32768 rows
N_CORES = 8
N_SHARD = N_TOTAL // N_CORES   # 4096 rows per core
CH = 512                       # phase-2 row chunk (moving dim)
N_CHUNKS = N_SHARD // CH       # 8
N_SEG = 4                      # x^T DMA-transpose segments (per feature block: 2)
SEG = N_SHARD // 2             # 2048 rows per (block, half) segment

F32 = mybir.dt.float32
F16 = mybir.dt.float16
USE_RDMA = False


def _host_prep(base_weight, spline_weight, spline_scaler,
               bn_base_gamma, bn_base_beta, bn_base_mean, bn_base_var,
               bn_spline_gamma, bn_spline_beta, bn_spline_mean, bn_spline_var):
    """Fold BN + rewrite spline into relu-plane weights. All in float64.

    Returns SBUF-layout arrays:
      w_t  [128, 2, 2, 128]   (i', b, bo, o')  t-plane weights
      w_r  [128, 3, 2, 2, 128] (i', m, b, bo, o') relu-plane weights
      w_b  [128, 2, 2, 128]   base weights
      cs   [1, 256]           spline constant (bo-major)
      bb   [128, 2]           base bias per (o', bo), f32
    """
    f64 = np.float64
    w = np.asarray(spline_weight, f64) * np.asarray(spline_scaler, f64)[:, :, None]
    knots = np.linspace(-1.0, 1.0, K_KNOTS).astype(f64)
    jg = np.arange(5, dtype=f64) / 4.0
    tri = np.maximum(0.0, 1.0 - np.abs(jg[None, :] - knots[:, None]))   # [k, j]
    G = np.einsum('oik,kj->oij', w, tri)                                # [o,i,5]
    a_s = np.asarray(bn_spline_gamma, f64) / np.sqrt(np.asarray(bn_spline_var, f64) + EPS_BN)
    b_s = np.asarray(bn_spline_beta, f64) - a_s * np.asarray(bn_spline_mean, f64)
    G = G * a_s[:, None, None]
    W_t = (G[:, :, 1] - G[:, :, 0]).T                                   # [i,o]
    Hs = [(G[:, :, 2] - 2 * G[:, :, 1] + G[:, :, 0]).T,
          (G[:, :, 3] - 2 * G[:, :, 2] + G[:, :, 1]).T,
          (G[:, :, 4] - 2 * G[:, :, 3] + G[:, :, 2]).T]                 # [i,o]
    C_s = G[:, :, 0].sum(axis=1) + b_s                                  # [o]
    a_b = np.asarray(bn_base_gamma, f64) / np.sqrt(np.asarray(bn_base_var, f64) + EPS_BN)
    b_b = np.asarray(bn_base_beta, f64) - a_b * np.asarray(bn_base_mean, f64)
    Wb = np.asarray(base_weight, f64) * a_b[None, :]                    # [i,o]

    def blk(M):  # [in, out] f64 -> [128, 2(b), 2(bo), 128] (i', b, bo, o')
        return M.reshape(2, 128, 2, 128).transpose(1, 0, 2, 3)

    w_t = blk(W_t).astype(np.float16)
    w_b = blk(Wb).astype(np.float16)
    w_r = np.stack([blk(Hm) for Hm in Hs], axis=1).astype(np.float16)  # [128,3,2,2,128]
    cs = C_s.reshape(2, 128).T.astype(np.float32)                      # [128, 2]
    bb = b_b.reshape(2, 128).T.astype(np.float32)                      # [128, 2]
    return w_t, w_r, w_b, cs, bb


def _build_bass(use_pool_epilogue=False):
    nc = bacc.Bacc(num_devices=N_CORES)
    x_sh = nc.declare_dram_parameter("x_sh", [N_SHARD, IN_F], F16, isOutput=False)
    w_t_d = nc.declare_dram_parameter("w_t", [128, 2, 2, 128], F16, isOutput=False)
    w_r_d = nc.declare_dram_parameter("w_r", [128, 3, 2, 2, 128], F16, isOutput=False)
    w_b_d = nc.declare_dram_parameter("w_b", [128, 2, 2, 128], F16, isOutput=False)
    cs_d = nc.declare_dram_parameter("cs", [128, 2], F32, isOutput=False)
    bb_d = nc.declare_dram_parameter("bb", [128, 2], F32, isOutput=False)
    out_t = nc.declare_dram_parameter("out_t", [2, 128, N_SHARD], F16, isOutput=True)

    from contextlib import ExitStack
    with tile.TileContext(nc) as tc, ExitStack() as es:
        cons = es.enter_context(tc.tile_pool(name="cons", bufs=1))
        silu_p = es.enter_context(tc.tile_pool(name="silu", bufs=1))
        planes_p = es.enter_context(tc.tile_pool(name="planes", bufs=2))
        psS = es.enter_context(tc.tile_pool(name="psS", bufs=6, space="PSUM"))
        psB = es.enter_context(tc.tile_pool(name="psB", bufs=2, space="PSUM"))
        outp = es.enter_context(tc.tile_pool(name="outp", bufs=3))
        dram = es.enter_context(tc.tile_pool(name="dram", bufs=2, space="DRAM"))

        # ---- weight loads (scalar/ACT queue; x transposes go on sync) ----
        wb_sb = cons.tile([128, 2, 2, 128], F16, name="wb_sb")
        nc.scalar.dma_start(out=wb_sb[:], in_=w_b_d[:])
        wt_sb = cons.tile([128, 2, 2, 128], F16, name="wt_sb")
        nc.scalar.dma_start(out=wt_sb[:], in_=w_t_d[:])
        wr_sb = cons.tile([128, 3, 2, 2, 128], F16, name="wr_sb")
        nc.scalar.dma_start(out=wr_sb[:], in_=w_r_d[:])
        cs_sb = cons.tile([128, 2], F32, name="cs_sb")
        nc.scalar.dma_start(out=cs_sb[:], in_=cs_d[:])
        bb_sb = cons.tile([128, 2], F32, name="bb_sb")
        nc.scalar.dma_start(out=bb_sb[:], in_=bb_d[:])

        rb = cons.tile([128, 2], F32, name="rb")     # ACT Relu biases -1, -2
        nc.vector.memset(rb[:, 0:1], -1.0)
        nc.vector.memset(rb[:, 1:2], -2.0)

        # ---- phase 1: x^T via DMA XBAR transpose, segmented minmax ----
        xt = cons.tile([128, 2, N_SHARD], F16, name="xt")
        for b in range(2):
            for h in range(2):
                nc.sync.dma_start(
                    out=xt[:, b, h * SEG:(h + 1) * SEG],
                    in_=x_sh[h * SEG:(h + 1) * SEG, b * 128:(b + 1) * 128],
                    transpose=True)

        # local min/max: block 0 reduces on Pool, block 1 on DVE (parallel)
        pmin = cons.tile([128, 2, 2], F32, name="pmin")   # (b, h)
        pmax = cons.tile([128, 2, 2], F32, name="pmax")
        for b in range(2):
            eng = nc.vector
            for h in range(2):
                seg = xt[:, b, h * SEG:(h + 1) * SEG]
                eng.tensor_reduce(
                    out=pmin[:, b, h:h + 1], in_=seg,
                    op=mybir.AluOpType.min, axis=mybir.AxisListType.X)
                eng.tensor_reduce(
                    out=pmax[:, b, h:h + 1], in_=seg,
                    op=mybir.AluOpType.max, axis=mybir.AxisListType.X)
        mm_loc = cons.tile([128, 4], F32, name="mm_loc")  # [min0,min1,-max0,-max1]
        tmax = cons.tile([128, 2], F32, name="tmax")
        nc.vector.tensor_tensor(
            out=mm_loc[:, 0:2], in0=pmin[:, :, 0], in1=pmin[:, :, 1],
            op=mybir.AluOpType.min)
        nc.vector.tensor_tensor(
            out=tmax[:], in0=pmax[:, :, 0], in1=pmax[:, :, 1],
            op=mybir.AluOpType.max)
        nc.vector.tensor_scalar(
            out=mm_loc[:, 2:4], in0=tmax[:], scalar1=-1.0, scalar2=None,
            op0=mybir.AluOpType.mult)

        # ---- global min/max across the 8 cores ----
        if USE_RDMA:
            # XOR recursive doubling over the 8 same-device cores with raw
            # p2p remote DMA: no CC cores, no runtime collective barrier.
            # Round k: send acc to peer (tpb ^= 1<<k); wait for the peer's
            # symmetric write (+2 on rsem[k], one sem per round so a fast
            # core's later round can't satisfy an earlier wait); combine
            # with elementwise min.  [min|-max] carries max via min.
            # Hand-synced inside tile_critical (scheduler can't model the
            # cross-core sem arrivals).
            rsem = [nc.alloc_semaphore(f"xmm{k}") for k in range(3)]
            psem = nc.alloc_semaphore("xmm_prep")
            xch = nc.alloc_semaphore("xmm_comb")
            loc_sem = nc.alloc_semaphore("rdma_local")
            XW = 128   # pad payload to 512B/partition for the DMA ucode
            rx = [cons.tile([128, XW], F32, name=f"rx{k}") for k in range(3)]
            accs = [cons.tile([128, XW], F32, name=f"acc{k}") for k in range(4)]
            nc.vector.memset(accs[0][:, 4:XW], 0.0)
            nc.vector.tensor_copy(out=accs[0][:, 0:4], in_=mm_loc[:])
            with tc.tile_critical():
                for k in range(3):
                    rdests = [None] * 8
                    rdests[4 if (1 << k) & 4 else 0] = (0, 1 << k)
                    prep = nc.gpsimd.remote_dma_broadcast(
                        out_ap=rx[k][:], in_ap=accs[k][:],
                        remote_sem=rsem[k], local_sem=loc_sem,
                        rdests=rdests)
                    if k > 0:
                        prep._wait_ge(xch, k)  # Pool in-order: gates trigger too
                    prep.then_inc(psem, 1)
                    nc.gpsimd.trigger_dma(count=1)._wait_ge(psem, k + 1)
                    comb = nc.vector.tensor_tensor(
                        out=accs[k + 1][:], in0=accs[k][:], in1=rx[k][:],
                        op=mybir.AluOpType.min)
                    comb._wait_ge(rsem[k], 2)
                    comb.then_inc(xch, 1)
            gmm = accs[3]   # [gmin0,gmin1,-gmax0,-gmax1] in cols 0:4
        else:
            cc_in = dram.tile([128, 4], F32)
            cc_out = dram.tile([128, 4], F32)
            nc.sync.dma_start(out=cc_in[:], in_=mm_loc[:])
            nc.gpsimd.collective_compute(
                "AllReduce", mybir.AluOpType.min,
                replica_groups=[list(range(N_CORES))],
                ins=[cc_in.opt()], outs=[cc_out.opt()])
            gmm = cons.tile([128, 4], F32, name="gmm")
            nc.sync.dma_start(out=gmm[:], in_=cc_out[:])

        # s4 = 4/(gmax-gmin+eps)
        nrng = cons.tile([128, 2], F32, name="nrng")
        qt = cons.tile([128, 2], F32, name="qt")
        s4 = cons.tile([128, 2], F32, name="s4")
        nc.vector.tensor_tensor(
            out=nrng[:], in0=gmm[:, 0:2], in1=gmm[:, 2:4],
            op=mybir.AluOpType.add)                       # gmin - gmax
        nc.vector.tensor_scalar(
            out=qt[:], in0=nrng[:], scalar1=-0.25, scalar2=EPS_MINMAX * 0.25,
            op0=mybir.AluOpType.mult, op1=mybir.AluOpType.add)
        nc.vector.reciprocal(out=s4[:], in_=qt[:])

        # ---- phase 1.5: base path (x @ Wb, silu w/ per-partition bias) ----
        # runs on PE/ACT while the collective is in flight.
        silu_sb = [[silu_p.tile([128, CH], F16, name=f"silu_{c}_{bo}")
                    for bo in range(2)] for c in range(N_CHUNKS)]
        for c in range(N_CHUNKS):
            cs_sl = slice(c * CH, (c + 1) * CH)
            for bo in range(2):
                pb = psB.tile([128, CH], F32)
                for b in range(2):
                    nc.tensor.matmul(
                        pb[:], wb_sb[:, b, bo, :], xt[:, b, cs_sl],
                        start=(b == 0), stop=(b == 1), skip_group_check=True)
                nc.scalar.activation(
                    out=silu_sb[c][bo][:], in_=pb[:],
                    func=mybir.ActivationFunctionType.Silu,
                    bias=bb_sb[:, bo:bo + 1], scale=1.0)
                # fold the spline constant C_s in here (idle window) so the
                # spline groups don't need a rank-1 bias matmul
                nc.vector.tensor_scalar(
                    out=silu_sb[c][bo][:], in0=silu_sb[c][bo][:],
                    scalar1=cs_sb[:, bo:bo + 1], scalar2=None,
                    op0=mybir.AluOpType.add)

        # ---- phase 2: planes + spline GEMMs + epilogue ----
        for c in range(N_CHUNKS):
            cs_sl = slice(c * CH, (c + 1) * CH)
            tpl = [planes_p.tile([128, CH], F16, tag=f"t{b}", name=f"t{b}_{c}")
                   for b in range(2)]
            rpl = [[planes_p.tile([128, CH], F16, tag=f"r{m}{b}", name=f"r{m}{b}_{c}")
                    for b in range(2)] for m in range(3)]
            for b in range(2):
                nc.vector.tensor_scalar(
                    out=tpl[b][:], in0=xt[:, b, cs_sl],
                    scalar1=gmm[:, b:b + 1], scalar2=s4[:, b:b + 1],
                    op0=mybir.AluOpType.subtract, op1=mybir.AluOpType.mult)
                for m in (1, 2):
                    nc.scalar.activation(
                        out=rpl[m - 1][b][:], in_=tpl[b][:],
                        func=mybir.ActivationFunctionType.Relu,
                        bias=rb[:, m - 1:m], scale=1.0)
                nc.vector.tensor_scalar(
                    out=rpl[2][b][:], in0=tpl[b][:], scalar1=3.0, scalar2=0.0,
                    op0=mybir.AluOpType.subtract, op1=mybir.AluOpType.max)
            for bo in range(2):
                ps = psS.tile([128, CH], F32)
                for b in range(2):
                    nc.tensor.matmul(
                        ps[:], wt_sb[:, b, bo, :], tpl[b][:],
                        start=(b == 0), stop=False, skip_group_check=True)
                for m in range(3):
                    for b in range(2):
                        nc.tensor.matmul(
                            ps[:], wr_sb[:, m, b, bo, :], rpl[m][b][:],
                            start=False, stop=(m == 2 and b == 1),
                            skip_group_check=True)
                o = outp.tile([128, CH], F16)
                eng = nc.gpsimd if use_pool_epilogue else nc.vector
                eng.tensor_tensor(
                    out=o[:], in0=ps[:], in1=silu_sb[c][bo][:],
                    op=mybir.AluOpType.add)
                nc.sync.dma_start(out=out_t[bo, :, cs_sl], in_=o[:])
    nc.compile()
    return nc


_CACHE = {}


def make_in_maps(inputs):
    x = np.asarray(inputs["x"], np.float32).reshape(N_TOTAL, IN_F)
    x16 = x.astype(np.float16)
    w_t, w_r, w_b, cs, bb = _host_prep(
        **{k: v for k, v in inputs.items() if k != "x"})
    return [{
        "x_sh": np.ascontiguousarray(x16[c * N_SHARD:(c + 1) * N_SHARD]),
        "w_t": w_t, "w_r": w_r, "w_b": w_b, "cs": cs, "bb": bb,
    } for c in range(N_CORES)]


def kernel(**inputs):
    if "nc" not in _CACHE:
        _CACHE["nc"] = _build_bass()
    nc = _CACHE["nc"]
    in_maps = make_in_maps(inputs)
    res = run_bass_kernel_spmd(nc, in_maps, list(range(N_CORES)))
    out = np.empty((N_TOTAL, OUT_F), np.float32)
    for c in range(N_CORES):
        ot = np.asarray(res.results[c]["out_t"], np.float32)  # [2,128,4096]
        out[c * N_SHARD:(c + 1) * N_SHARD] = (
            ot.transpose(2, 0, 1).reshape(N_SHARD, OUT_F))
    return out.reshape(B, H, W, OUT_F)
